# revision 10
# baseline (speedup 1.0000x reference)
"""VQ codebook (K-means batch) loss kernel for 8 Trainium2 NeuronCores.

loss = mean((quantize(x) - x)^2)
     = (sum(x^2) + sum_rows min_k(||w_k||^2 - 2 x.w_k)) / (N*D)

Sharding: data-parallel over flattened N (4096 rows/core), codebook replicated.

Per core:
  - SWDGE DMA casts fp32 -> fp8e4 on the fly while loading x (4 big loads,
    ramped sizes so compute starts early)
  - PE: fp8 DoubleRow matmuls (256-deep contraction, 0.5 cyc/row):
      psum[128 rows, 1024 k] = wsq_k - 2 x.w_k
    wsq enters via one extra DoubleRow matmul whose 4 constant contraction
    rows are a multi-scale fp8 decomposition wsq ~= sum_j s_j * v_j[k]
    (s = [64, 8, 1, 1/8]; residual < 0.008).  ~30 warmup matmuls on a
    zeroed tile ramp the PE p-state while the first DMAs land.
  - row-min hybrid (a DVE instruction may read at most ONE PSUM operand,
    so the work is split across DVE and ACT):
      R1 tiles: DVE tensor_reduce(min) straight over [128, 1024] PSUM
      R2 tiles: ACT copies PSUM -> fp16 SBUF, DVE finishes with a
                min/min tensor_tensor_scan + last-column copy
  - sum(x^2): ACT Square+accum pieces interleaved between the R2 copies;
    one piece runs as a DVE scalar_tensor_tensor during its idle startup
Host sums the tiny per-core outputs in fp64.
"""

import numpy as np
import ml_dtypes
from contextlib import ExitStack

import concourse.bass as bass
import concourse.tile as tile
from concourse import bacc, mybir
from concourse.bass_utils import run_bass_kernel_spmd

N_CORES = 8
D = 512           # embedding dim
K = 1024          # codebook size
R_TOT = 64 * 512  # total rows
R = R_TOT // N_CORES  # rows per core = 4096
CH = D // 128      # 4 contraction chunks
LOAD_ROWS = [512, 512, 1024, 2048]  # ramped prefetch: small first loads
NL = len(LOAD_ROWS)
LOAD_OFF = [sum(LOAD_ROWS[:i]) for i in range(NL)]
M_TOT = R // 128   # 32

BIG = 3.0e38
FP8 = mybir.dt.float8e4
DR = mybir.MatmulPerfMode.DoubleRow

_CACHE = {}


def _build():
    if "nc" in _CACHE:
        return _CACHE["nc"]
    nc = bacc.Bacc(
        "TRN2",
        target_bir_lowering=False,
        debug=False,
        enable_asserts=False,
        num_devices=N_CORES,
    )
    xq = nc.dram_tensor("xq", [128, CH, R], mybir.dt.float32, kind="ExternalInput").ap()
    wq = nc.dram_tensor("wq", [128, CH, K], FP8, kind="ExternalInput").ap()
    wvs = nc.dram_tensor("wvs", [2, 2, K + 128], FP8, kind="ExternalInput").ap()
    rm = nc.dram_tensor("rm", [128, M_TOT], mybir.dt.float32, kind="ExternalOutput").ap()
    xsq = nc.dram_tensor("xsq", [128, 5], mybir.dt.float32, kind="ExternalOutput").ap()

    with tile.TileContext(nc) as tc, ExitStack() as ctx:
        wpool = ctx.enter_context(tc.tile_pool(name="w", bufs=1))
        xpool = ctx.enter_context(tc.tile_pool(name="xb", bufs=NL))
        cpool = ctx.enter_context(tc.tile_pool(name="cp", bufs=3))
        spool = ctx.enter_context(tc.tile_pool(name="sc", bufs=2))
        qpool = ctx.enter_context(tc.tile_pool(name="sq", bufs=2))
        opool = ctx.enter_context(tc.tile_pool(name="outs", bufs=1))
        ppool = ctx.enter_context(tc.tile_pool(name="ps", bufs=4, space="PSUM"))

        w_s = wpool.tile([128, CH, K], FP8)
        wvs_s = wpool.tile([2, 2, K + 128], FP8)
        wsqv_s = wvs_s[:, :, 0:K]
        ss_s = wvs_s[:, :, K : K + 128]
        # issue order tuned for the DMA_ENGINES serial queue: first k-half of
        # the codebook, tiny wsq rows, then the second k-half
        nc.sync.dma_start(out=w_s[:, :, 0:512], in_=wq[:, :, 0:512])
        nc.sync.dma_start(out=wvs_s[:], in_=wvs[:, :, :])
        nc.sync.dma_start(out=w_s[:, :, 512:1024], in_=wq[:, :, 512:1024])

        rm_s = opool.tile([128, M_TOT], mybir.dt.float32)
        xsq_s = opool.tile([128, 5], mybir.dt.float32)

        # PE p-state warmup: dummy matmuls on a zeroed tile keep the PE busy
        # while the first DMAs land, so real matmuls run at full clock.
        import os as _os
        WARM_BIG = int(_os.environ.get("KWARM_BIG", "25"))
        WARM_SMALL = int(_os.environ.get("KWARM_SMALL", "10"))
        if WARM_BIG or WARM_SMALL:
            zz = wpool.tile([2, 128], mybir.dt.bfloat16)
            nc.vector.memset(zz[:], 0.0)
            wps = ppool.tile([128, K], mybir.dt.float32, tag="ps", name="warm_ps")
            for wi in range(WARM_BIG):
                nc.tensor.matmul(
                    wps[:, 0:128], lhsT=zz[:, 0:128], rhs=zz[:, 0:128],
                    start=True, stop=True,
                )
            for wi in range(WARM_SMALL):
                nc.tensor.matmul(
                    wps[:, 0:16], lhsT=zz[:, 0:128], rhs=zz[:, 0:16],
                    start=True, stop=True,
                )

        # all x loads issued up front; SWDGE casts fp32 -> fp8 in the DMA
        xb = []
        for l in range(NL):
            rl = LOAD_ROWS[l]
            t = xpool.tile([128, CH, rl], FP8, tag=f"xb{l}", name=f"xb_{l}")
            nc.gpsimd.dma_start(
                out=t[:], in_=xq[:, :, LOAD_OFF[l] : LOAD_OFF[l] + rl]
            )
            xb.append(t)

        ps_t = {}

        def fill(l, mm, m, half):
            rsl = slice(mm * 128, (mm + 1) * 128)
            if half == 0:
                ps_t[m] = ppool.tile(
                    [128, K], mybir.dt.float32, tag="ps", name=f"ps_{m}"
                )
            ps = ps_t[m]
            sl = slice(half * 512, (half + 1) * 512)
            nc.tensor.matmul(
                ps[:, sl], lhsT=xb[l][:, 0:2, rsl], rhs=w_s[:, 0:2, sl],
                start=True, stop=False, perf_mode=DR,
            )
            nc.tensor.matmul(
                ps[:, sl], lhsT=xb[l][:, 2:4, rsl], rhs=w_s[:, 2:4, sl],
                start=False, stop=False, perf_mode=DR,
            )
            nc.tensor.matmul(
                ps[:, sl], lhsT=ss_s, rhs=wsqv_s[:, :, sl],
                start=False, stop=True, perf_mode=DR,
            )

        import os as _os2
        R2_START = int(_os2.environ.get("KR2S", "0"))
        R2_SET = set(range(R2_START, R2_START + 2 * int(_os2.environ.get("KR2", "15")), 2))

        def reduce_min(m):
            ps = ps_t.pop(m)
            if m in R2_SET:
                # ACT moves the distances to SBUF fp16; DVE finishes with a
                # min/min scan (PSUM can appear in at most one DVE operand,
                # so the one-pass dual-PSUM reduce is not available)
                cp = cpool.tile(
                    [128, 1024], mybir.dt.float16, tag="cp", name=f"cp_{m}"
                )
                nc.scalar.activation(
                    out=cp[:], in_=ps[:], func=mybir.ActivationFunctionType.Copy
                )
                sc = spool.tile(
                    [128, 512], mybir.dt.float16, tag="sc", name=f"sc_{m}"
                )
                nc.vector.tensor_tensor_scan(
                    out=sc[:],
                    data0=cp[:, 0:512],
                    data1=cp[:, 512:1024],
                    initial=60000.0,
                    op0=mybir.AluOpType.min,
                    op1=mybir.AluOpType.min,
                )
                nc.vector.tensor_copy(
                    out=rm_s[:, m : m + 1], in_=sc[:, 511:512]
                )
            else:
                nc.vector.tensor_reduce(
                    out=rm_s[:, m : m + 1], in_=ps[:],
                    axis=mybir.AxisListType.X, op=mybir.AluOpType.min,
                )

        # sum(x^2): 8 pieces of [128, CH, 512 rows], interleaved into ACT's
        # queue between distance copies so ACT stays busy without a single
        # long square blocking the copies the DVE scans depend on
        # fewer, larger square pieces amortize the per-instruction ramp and
        # accum-flush overhead on ACT (~10% cheaper per element)
        pieces = [(0, 0, 512), (1, 0, 512), (2, 0, 1024), (3, 0, 1024), (3, 1024, 1024)]
        piece_at = {8: 1, 14: 2, 20: 3, 26: 4}

        def emit_piece(k):
            l, off, n = pieces[k]
            sq = qpool.tile(
                [128, CH, n], mybir.dt.bfloat16, tag="sq", name=f"sq_{k}"
            )
            nc.scalar.activation(
                out=sq[:],
                in_=xb[l][:, :, off : off + n],
                func=mybir.ActivationFunctionType.Square,
                accum_out=xsq_s[:, k : k + 1],
            )

        # piece 0 on the DVE: it is otherwise idle until the first psum is
        # ready, and this frees ACT for two more distance copies
        sq0 = qpool.tile([128, CH, 512], mybir.dt.bfloat16, tag="sq", name="sq_dve")
        nc.vector.scalar_tensor_tensor(
            out=sq0[:],
            in0=xb[0][:, :, 0:512],
            scalar=1.0,
            in1=xb[0][:, :, 0:512],
            op0=mybir.AluOpType.mult,
            op1=mybir.AluOpType.mult,
            accum_out=xsq_s[:, 0:1],
        )

        m = 0
        for l in range(NL):
            n_mm = LOAD_ROWS[l] // 128
            for mm in range(n_mm):
                fill(l, mm, m + mm, 0)
                fill(l, mm, m + mm, 1)
                reduce_min(m + mm)
                if m + mm == 27:
                    # bulk of the row-min output leaves early; only the last
                    # four columns ride the critical tail
                    nc.sync.dma_start(out=rm[:, 0:28], in_=rm_s[:, 0:28])
                if m + mm == 30:
                    # sum(x^2) is complete once the last square piece retires;
                    # store it here so it does not serialize behind the final
                    # row-min store on the SP DGE path
                    nc.sync.dma_start(out=xsq[:, :], in_=xsq_s[:])
                if m + mm in piece_at:
                    emit_piece(piece_at[m + mm])
            m += n_mm

        nc.sync.dma_start(out=rm[:, 28:32], in_=rm_s[:, 28:32])

    nc.compile()
    _CACHE["nc"] = nc
    return nc


def _fp8(a):
    return a.astype(ml_dtypes.float8_e4m3)


def _prep(inputs, weight):
    x = np.asarray(inputs, dtype=np.float32).reshape(-1, D)  # [32768, 512]
    w = np.asarray(weight, dtype=np.float32)  # [1024, 512]

    # wq[p, c, k] = fp8(-2 * w[k, c*128+p])
    wq = _fp8(
        np.ascontiguousarray((-2.0 * w.T).reshape(CH, 128, K).transpose(1, 0, 2))
    )

    # multi-scale fp8 decomposition of wsq: wsq ~= sum_j s_j * v_j
    wsq = (w.astype(np.float64) ** 2).sum(axis=1).astype(np.float32)  # [1024]
    scales = [64.0, 8.0, 1.0, 0.125]
    res = wsq.copy()
    vs = []
    for s in scales:
        v = _fp8(res / s)
        vs.append(v)
        res = res - s * v.astype(np.float32)
    # wvs[p, i, 0:K] = v_{2i+p};  wvs[p, i, K:K+128] = s_{2i+p}
    wvs = np.zeros((2, 2, K + 128), dtype=ml_dtypes.float8_e4m3)
    for j, (s, v) in enumerate(zip(scales, vs)):
        p, i = j % 2, j // 2
        wvs[p, i, 0:K] = v
        wvs[p, i, K:] = s

    in_maps = []
    for c in range(N_CORES):
        shard = x[c * R : (c + 1) * R]  # [4096, 512]
        # xq[p, ch, n] = shard[n, ch*128+p]
        xqc = np.ascontiguousarray(shard.reshape(R, CH, 128).transpose(2, 1, 0))
        in_maps.append({"xq": xqc, "wq": wq, "wvs": wvs})
    return in_maps


def _run(inputs, weight, trace=False, **kw):
    nc = _build()
    in_maps = _prep(inputs, weight)
    res = run_bass_kernel_spmd(nc, in_maps, list(range(N_CORES)), trace=trace, **kw)
    total = 0.0
    for r in res.results:
        total += r["rm"].astype(np.float64).sum()
        total += r["xsq"].astype(np.float64).sum()
    loss = total / (R_TOT * D)
    return np.array(loss, dtype=np.float32), res


def kernel(inputs, weight):
    return _run(inputs, weight)[0]


# revision 11
# speedup vs baseline: 1.0009x; 1.0009x over previous
"""VQ codebook (K-means batch) loss kernel for 8 Trainium2 NeuronCores.

loss = mean((quantize(x) - x)^2)
     = (sum(x^2) + sum_rows min_k(||w_k||^2 - 2 x.w_k)) / (N*D)

Sharding: data-parallel over flattened N (4096 rows/core), codebook replicated.

Per core:
  - SWDGE DMA casts fp32 -> fp8e4 on the fly while loading x (4 big loads,
    ramped sizes so compute starts early)
  - PE: fp8 DoubleRow matmuls (256-deep contraction, 0.5 cyc/row):
      psum[128 rows, 1024 k] = wsq_k - 2 x.w_k
    wsq enters via one extra DoubleRow matmul whose 4 constant contraction
    rows are a multi-scale fp8 decomposition wsq ~= sum_j s_j * v_j[k]
    (s = [64, 8, 1, 1/8]; residual < 0.008).  ~30 warmup matmuls on a
    zeroed tile ramp the PE p-state while the first DMAs land.
  - row-min hybrid (a DVE instruction may read at most ONE PSUM operand,
    so the work is split across DVE and ACT):
      R1 tiles: DVE tensor_reduce(min) straight over [128, 1024] PSUM
      R2 tiles: ACT copies PSUM -> fp16 SBUF, DVE finishes with a
                min/min tensor_tensor_scan + last-column copy
  - sum(x^2): ACT Square+accum pieces interleaved between the R2 copies;
    one piece runs as a DVE scalar_tensor_tensor during its idle startup
Host sums the tiny per-core outputs in fp64.
"""

import numpy as np
import ml_dtypes
from contextlib import ExitStack

import concourse.bass as bass
import concourse.tile as tile
from concourse import bacc, mybir
from concourse.bass_utils import run_bass_kernel_spmd

N_CORES = 8
D = 512           # embedding dim
K = 1024          # codebook size
R_TOT = 64 * 512  # total rows
R = R_TOT // N_CORES  # rows per core = 4096
CH = D // 128      # 4 contraction chunks
LOAD_ROWS = [512, 512, 1024, 2048]  # ramped prefetch: small first loads
NL = len(LOAD_ROWS)
LOAD_OFF = [sum(LOAD_ROWS[:i]) for i in range(NL)]
M_TOT = R // 128   # 32

BIG = 3.0e38
FP8 = mybir.dt.float8e4
DR = mybir.MatmulPerfMode.DoubleRow

_CACHE = {}


def _build():
    if "nc" in _CACHE:
        return _CACHE["nc"]
    nc = bacc.Bacc(
        "TRN2",
        target_bir_lowering=False,
        debug=False,
        enable_asserts=False,
        num_devices=N_CORES,
    )
    xq = nc.dram_tensor("xq", [128, CH, R], mybir.dt.float32, kind="ExternalInput").ap()
    wq = nc.dram_tensor("wq", [128, CH, K], FP8, kind="ExternalInput").ap()
    wvs = nc.dram_tensor("wvs", [2, 2, K + 128], FP8, kind="ExternalInput").ap()
    rm = nc.dram_tensor("rm", [128, M_TOT], mybir.dt.float32, kind="ExternalOutput").ap()
    xsq = nc.dram_tensor("xsq", [128, 5], mybir.dt.float32, kind="ExternalOutput").ap()

    with tile.TileContext(nc) as tc, ExitStack() as ctx:
        wpool = ctx.enter_context(tc.tile_pool(name="w", bufs=1))
        xpool = ctx.enter_context(tc.tile_pool(name="xb", bufs=NL))
        cpool = ctx.enter_context(tc.tile_pool(name="cp", bufs=3))
        spool = ctx.enter_context(tc.tile_pool(name="sc", bufs=2))
        qpool = ctx.enter_context(tc.tile_pool(name="sq", bufs=2))
        opool = ctx.enter_context(tc.tile_pool(name="outs", bufs=1))
        ppool = ctx.enter_context(tc.tile_pool(name="ps", bufs=4, space="PSUM"))

        w_s = wpool.tile([128, CH, K], FP8)
        wvs_s = wpool.tile([2, 2, K + 128], FP8)
        wsqv_s = wvs_s[:, :, 0:K]
        ss_s = wvs_s[:, :, K : K + 128]
        # issue order tuned for the DMA_ENGINES serial queue: first k-half of
        # the codebook, tiny wsq rows, then the second k-half
        nc.sync.dma_start(out=w_s[:, :, 0:512], in_=wq[:, :, 0:512])
        nc.sync.dma_start(out=wvs_s[:], in_=wvs[:, :, :])
        nc.sync.dma_start(out=w_s[:, :, 512:1024], in_=wq[:, :, 512:1024])

        rm_s = opool.tile([128, M_TOT], mybir.dt.float32)
        xsq_s = opool.tile([128, 5], mybir.dt.float32)

        # PE p-state warmup: dummy matmuls on a zeroed tile keep the PE busy
        # while the first DMAs land, so real matmuls run at full clock.
        import os as _os
        WARM_BIG = int(_os.environ.get("KWARM_BIG", "25"))
        WARM_SMALL = int(_os.environ.get("KWARM_SMALL", "10"))
        if WARM_BIG or WARM_SMALL:
            zz = wpool.tile([2, 128], mybir.dt.bfloat16)
            nc.vector.memset(zz[:], 0.0)
            wps = ppool.tile([128, K], mybir.dt.float32, tag="ps", name="warm_ps")
            for wi in range(WARM_BIG):
                nc.tensor.matmul(
                    wps[:, 0:128], lhsT=zz[:, 0:128], rhs=zz[:, 0:128],
                    start=True, stop=True,
                )
            for wi in range(WARM_SMALL):
                nc.tensor.matmul(
                    wps[:, 0:16], lhsT=zz[:, 0:128], rhs=zz[:, 0:16],
                    start=True, stop=True,
                )

        # all x loads issued up front; SWDGE casts fp32 -> fp8 in the DMA
        xb = []
        for l in range(NL):
            rl = LOAD_ROWS[l]
            t = xpool.tile([128, CH, rl], FP8, tag=f"xb{l}", name=f"xb_{l}")
            nc.gpsimd.dma_start(
                out=t[:], in_=xq[:, :, LOAD_OFF[l] : LOAD_OFF[l] + rl]
            )
            xb.append(t)

        ps_t = {}

        def fill(l, mm, m, half):
            rsl = slice(mm * 128, (mm + 1) * 128)
            if half == 0:
                ps_t[m] = ppool.tile(
                    [128, K], mybir.dt.float32, tag="ps", name=f"ps_{m}"
                )
            ps = ps_t[m]
            sl = slice(half * 512, (half + 1) * 512)
            nc.tensor.matmul(
                ps[:, sl], lhsT=xb[l][:, 0:2, rsl], rhs=w_s[:, 0:2, sl],
                start=True, stop=False, perf_mode=DR,
            )
            nc.tensor.matmul(
                ps[:, sl], lhsT=xb[l][:, 2:4, rsl], rhs=w_s[:, 2:4, sl],
                start=False, stop=False, perf_mode=DR,
            )
            nc.tensor.matmul(
                ps[:, sl], lhsT=ss_s, rhs=wsqv_s[:, :, sl],
                start=False, stop=True, perf_mode=DR,
            )

        import os as _os2
        R2_START = int(_os2.environ.get("KR2S", "0"))
        R2_SET = set(range(R2_START, R2_START + 2 * int(_os2.environ.get("KR2", "15")), 2))

        def reduce_min(m):
            ps = ps_t.pop(m)
            if m in R2_SET:
                # ACT moves the distances to SBUF fp16; DVE finishes with a
                # min/min scan (PSUM can appear in at most one DVE operand,
                # so the one-pass dual-PSUM reduce is not available)
                cp = cpool.tile(
                    [128, 1024], mybir.dt.float16, tag="cp", name=f"cp_{m}"
                )
                nc.scalar.activation(
                    out=cp[:], in_=ps[:], func=mybir.ActivationFunctionType.Copy
                )
                # stride-0 output: the scan writes its running state to the
                # same column every step, so the final write IS the row min —
                # no scratch tile and no separate column copy
                nc.vector.tensor_tensor_scan(
                    out=rm_s[:, m : m + 1].broadcast_to([128, 512]),
                    data0=cp[:, 0:512],
                    data1=cp[:, 512:1024],
                    initial=60000.0,
                    op0=mybir.AluOpType.min,
                    op1=mybir.AluOpType.min,
                )
            else:
                nc.vector.tensor_reduce(
                    out=rm_s[:, m : m + 1], in_=ps[:],
                    axis=mybir.AxisListType.X, op=mybir.AluOpType.min,
                )

        # sum(x^2): 8 pieces of [128, CH, 512 rows], interleaved into ACT's
        # queue between distance copies so ACT stays busy without a single
        # long square blocking the copies the DVE scans depend on
        # fewer, larger square pieces amortize the per-instruction ramp and
        # accum-flush overhead on ACT (~10% cheaper per element)
        pieces = [(0, 0, 512), (1, 0, 512), (2, 0, 1024), (3, 0, 1024), (3, 1024, 1024)]
        piece_at = {8: 1, 14: 2, 20: 3, 26: 4}

        def emit_piece(k):
            l, off, n = pieces[k]
            sq = qpool.tile(
                [128, CH, n], mybir.dt.bfloat16, tag="sq", name=f"sq_{k}"
            )
            nc.scalar.activation(
                out=sq[:],
                in_=xb[l][:, :, off : off + n],
                func=mybir.ActivationFunctionType.Square,
                accum_out=xsq_s[:, k : k + 1],
            )

        # piece 0 on the DVE: it is otherwise idle until the first psum is
        # ready, and this frees ACT for two more distance copies
        sq0 = qpool.tile([128, CH, 512], mybir.dt.bfloat16, tag="sq", name="sq_dve")
        nc.vector.scalar_tensor_tensor(
            out=sq0[:],
            in0=xb[0][:, :, 0:512],
            scalar=1.0,
            in1=xb[0][:, :, 0:512],
            op0=mybir.AluOpType.mult,
            op1=mybir.AluOpType.mult,
            accum_out=xsq_s[:, 0:1],
        )

        m = 0
        for l in range(NL):
            n_mm = LOAD_ROWS[l] // 128
            for mm in range(n_mm):
                fill(l, mm, m + mm, 0)
                fill(l, mm, m + mm, 1)
                reduce_min(m + mm)
                if m + mm == 27:
                    # bulk of the row-min output leaves early; only the last
                    # four columns ride the critical tail
                    nc.sync.dma_start(out=rm[:, 0:28], in_=rm_s[:, 0:28])
                if m + mm == 30:
                    # sum(x^2) is complete once the last square piece retires;
                    # store it here so it does not serialize behind the final
                    # row-min store on the SP DGE path
                    nc.sync.dma_start(out=xsq[:, :], in_=xsq_s[:])
                if m + mm in piece_at:
                    emit_piece(piece_at[m + mm])
            m += n_mm

        nc.sync.dma_start(out=rm[:, 28:32], in_=rm_s[:, 28:32])

    nc.compile()
    _CACHE["nc"] = nc
    return nc


def _fp8(a):
    return a.astype(ml_dtypes.float8_e4m3)


def _prep(inputs, weight):
    x = np.asarray(inputs, dtype=np.float32).reshape(-1, D)  # [32768, 512]
    w = np.asarray(weight, dtype=np.float32)  # [1024, 512]

    # wq[p, c, k] = fp8(-2 * w[k, c*128+p])
    wq = _fp8(
        np.ascontiguousarray((-2.0 * w.T).reshape(CH, 128, K).transpose(1, 0, 2))
    )

    # multi-scale fp8 decomposition of wsq: wsq ~= sum_j s_j * v_j
    wsq = (w.astype(np.float64) ** 2).sum(axis=1).astype(np.float32)  # [1024]
    scales = [64.0, 8.0, 1.0, 0.125]
    res = wsq.copy()
    vs = []
    for s in scales:
        v = _fp8(res / s)
        vs.append(v)
        res = res - s * v.astype(np.float32)
    # wvs[p, i, 0:K] = v_{2i+p};  wvs[p, i, K:K+128] = s_{2i+p}
    wvs = np.zeros((2, 2, K + 128), dtype=ml_dtypes.float8_e4m3)
    for j, (s, v) in enumerate(zip(scales, vs)):
        p, i = j % 2, j // 2
        wvs[p, i, 0:K] = v
        wvs[p, i, K:] = s

    in_maps = []
    for c in range(N_CORES):
        shard = x[c * R : (c + 1) * R]  # [4096, 512]
        # xq[p, ch, n] = shard[n, ch*128+p]
        xqc = np.ascontiguousarray(shard.reshape(R, CH, 128).transpose(2, 1, 0))
        in_maps.append({"xq": xqc, "wq": wq, "wvs": wvs})
    return in_maps


def _run(inputs, weight, trace=False, **kw):
    nc = _build()
    in_maps = _prep(inputs, weight)
    res = run_bass_kernel_spmd(nc, in_maps, list(range(N_CORES)), trace=trace, **kw)
    total = 0.0
    for r in res.results:
        total += r["rm"].astype(np.float64).sum()
        total += r["xsq"].astype(np.float64).sum()
    loss = total / (R_TOT * D)
    return np.array(loss, dtype=np.float32), res


def kernel(inputs, weight):
    return _run(inputs, weight)[0]
